# revision 20
# baseline (speedup 1.0000x reference)
"""AdditiveRelationalGraphConvolution on 8 TRN2 NeuronCores.

out = relu(mean_s(features[neighbors]) @ W.T + mean_s(RWT[relations]))

Data-parallel over batch (4096 rows/core); feature table replicated (bf16).
Per 128-row tile:
  - neighbor rows are fetched with dma_gather (int16 indices). The 100K-row
    table exceeds int16 range, so samples are bucket-sorted (host side) into
    4 static windows of <=32768 rows; each bucket list is quota-padded with
    trailing -1 (skipped by HW, no traffic). Gathered slots land at
    dst[i%128, i//128]; a host-provided per-slot owner tag (batch row, or 255
    for dead slots) lets the device rebuild one-hot selection matrices
    (DVE is_equal) and aggregate with PE matmuls: aggT[i,b] += G[p,i]*sel[p,b].
  - relation rows (238-row table, int16-native) are gathered in natural
    (b,s)-slot order; a constant sel8 [128,8] matmul accumulates them straight
    into the output PSUM.
  - main transform: psum[b,o] = aggT.T @ (W.T/16) accumulated with the
    relation term, then relu on ACT, store.
"""

import sys

sys.path.insert(0, "/opt/trn_rl_repo")

import numpy as np

N_CORES = 8
B = 32768
S = 16
D = 256
NUM_NODES = 100000
NUM_REL = 238
NUM_PAIR = NUM_REL * (NUM_REL + 1) // 2  # 28441, fits int16
B_LOC = B // N_CORES  # 4096
P = 128
TILES = B_LOC // P  # 32

# feature-index windows (int16 range) and per-tile slot quotas
WIN = [(0, 32768), (32768, 65536), (65536, 98304), (98304, 100000)]
DEFAULT_QUOTA = (896, 896, 896, 128)  # multiples of 128
RELCHUNKS = (P * S // 2) // P  # 8 pair slots per tile chunked by 128
DEAD = 255.0

_CACHE = {}


def _build(QUOTA):
    import concourse.bass as bass
    import concourse.tile as tile
    from concourse import bacc, mybir
    from concourse.bass import IndirectOffsetOnAxis  # noqa: F401

    NCHUNK = [q // P for q in QUOTA]
    CHUNKS = sum(NCHUNK)
    f32 = mybir.dt.float32
    bf16 = mybir.dt.bfloat16
    i16 = mybir.dt.int16

    nc = bacc.Bacc(
        "TRN2",
        target_bir_lowering=False,
        debug=False,
        enable_asserts=False,
        num_devices=N_CORES,
        num_swdge_queues=4,
    )
    feat = nc.dram_tensor("feat", [NUM_NODES, D], bf16, kind="ExternalInput").ap()
    rwt = nc.dram_tensor("rwt", [NUM_PAIR, D], bf16, kind="ExternalInput").ap()
    wT = nc.dram_tensor("wT", [D, D], bf16, kind="ExternalInput").ap()
    # per-tile concatenated per-bucket wrapped int16 index lists
    IDXCOLS_G = sum(QUOTA) // 16  # 176 cols per tile
    IDXCOLS = IDXCOLS_G
    nidx = nc.dram_tensor("nidx", [P, TILES * IDXCOLS], i16, kind="ExternalInput").ap()
    ridx = nc.dram_tensor(
        "ridx", [P, TILES * (P * S // 32)], i16, kind="ExternalInput"
    ).ap()
    owner = nc.dram_tensor(
        "owner", [P, TILES * CHUNKS], f32, kind="ExternalInput"
    ).ap()
    iota = nc.dram_tensor("iota", [P, P], bf16, kind="ExternalInput").ap()
    selr = nc.dram_tensor("selr", [P, RELCHUNKS * P], bf16, kind="ExternalInput").ap()
    out = nc.dram_tensor("out", [B_LOC, D], f32, kind="ExternalOutput").ap()

    with tile.TileContext(nc) as tc:
        with (
            tc.tile_pool(name="const", bufs=1) as cp,
            tc.tile_pool(name="gfix", bufs=2) as gfix,
            tc.tile_pool(name="sel", bufs=2) as selp,
            tc.tile_pool(name="small", bufs=3) as small,
            tc.tile_pool(name="psA", bufs=2, space="PSUM") as psA,
            tc.tile_pool(name="psB", bufs=2, space="PSUM") as psB,
        ):
            wt_sb = cp.tile([P, 2 * D], bf16)
            nc.sync.dma_start(out=wt_sb[:, 0:D], in_=wT[0:P, :])
            nc.sync.dma_start(out=wt_sb[:, D : 2 * D], in_=wT[P : 2 * P, :])
            nidx_sb = cp.tile([P, TILES * IDXCOLS], i16)
            nc.sync.dma_start(out=nidx_sb[:], in_=nidx[:])
            ridx_sb = cp.tile([P, TILES * (P * S // 32)], i16)
            nc.sync.dma_start(out=ridx_sb[:], in_=ridx[:])
            owner_sb = cp.tile([P, TILES * CHUNKS], f32)
            nc.sync.dma_start(out=owner_sb[:], in_=owner[:])
            iota3_sb = cp.tile([P, P], bf16)
            nc.sync.dma_start(out=iota3_sb[:], in_=iota[:])
            selr_sb = cp.tile([P, RELCHUNKS * P], bf16)
            nc.sync.dma_start(out=selr_sb[:], in_=selr[:])

            qctr = [0]

            def nextq():
                q = qctr[0] % 4
                qctr[0] += 1
                return q

            GRP = 4  # tiles per gather group (quota-aligned concatenation)
            Gg = {}
            Rg = {}
            for tg in range(TILES // GRP):
                col0 = tg * GRP * IDXCOLS_G
                off = 0
                G = []
                for k in range(4):
                    w = GRP * QUOTA[k] // 16
                    g = gfix.tile(
                        [P, GRP * NCHUNK[k] * D],
                        bf16,
                        name=f"g{tg}_{k}",
                        tag=f"gath{k}",
                        bufs=3 if k < 3 else 2,
                    )
                    nc.gpsimd.dma_gather(
                        out_ap=g[:].rearrange("p (c d) -> p c d", d=D),
                        in_ap=feat[WIN[k][0] : WIN[k][1], :],
                        idxs_ap=nidx_sb[:, col0 + off : col0 + off + w],
                        num_idxs=GRP * QUOTA[k],
                        num_idxs_reg=GRP * QUOTA[k],
                        elem_size=D,
                        single_packet=False,
                        queue_num=nextq(),
                    )
                    off += w
                    G.append(g)
                rw = GRP * P * S // 32
                R = []
                for h in range(2):
                    rh = gfix.tile(
                        [P, GRP * RELCHUNKS * D // 2],
                        bf16,
                        name=f"r{tg}_{h}",
                        tag=f"gathr{h}",
                    )
                    nc.gpsimd.dma_gather(
                        out_ap=rh[:].rearrange("p (c d) -> p c d", d=D),
                        in_ap=rwt[:],
                        idxs_ap=ridx_sb[
                            :, tg * rw + h * rw // 2 : tg * rw + (h + 1) * rw // 2
                        ],
                        num_idxs=GRP * P * S // 4,
                        num_idxs_reg=GRP * P * S // 4,
                        elem_size=D,
                        single_packet=False,
                        queue_num=nextq(),
                    )
                    R.append(rh)
                Gg[tg] = G
                Rg[tg] = R

                for t in range(tg * GRP, (tg + 1) * GRP):
                    ti = t % GRP  # tile index within group

                    # all 22 one-hot selection matrices in one broadcast op:
                    # sel[p, c, b] = (owner[p, c] == iota[b])
                    sel = selp.tile([P, CHUNKS * P], bf16, tag="sel")
                    ow = owner_sb[:, t * CHUNKS : (t + 1) * CHUNKS]
                    nc.vector.tensor_tensor(
                        out=sel[:].rearrange("p (c b) -> p c b", b=P),
                        in0=ow[:, :, None].to_broadcast([P, CHUNKS, P]),
                        in1=iota3_sb[:, None, :].to_broadcast([P, CHUNKS, P]),
                        op=mybir.AluOpType.is_equal,
                    )

                    # aggT[i, b] = sum_p G[p, i] * sel[p, b] over all nbr chunks
                    agT0 = psA.tile([P, P], f32, tag="agT0", space="PSUM")
                    agT1 = psA.tile([P, P], f32, tag="agT1", space="PSUM")
                    ci = 0
                    for k in range(4):
                        for lc in range(NCHUNK[k]):
                            gc = ti * NCHUNK[k] + lc
                            for ic, agT in enumerate((agT0, agT1)):
                                nc.tensor.matmul(
                                    out=agT[:],
                                    lhsT=Gg[tg][k][
                                        :, gc * D + ic * P : gc * D + (ic + 1) * P
                                    ],
                                    rhs=sel[:, ci * P : (ci + 1) * P],
                                    start=(ci == 0),
                                    stop=(ci == CHUNKS - 1),
                                )
                            ci += 1
                    aggT = small.tile([P, 2 * P], bf16, tag="aggT")
                    nc.vector.tensor_copy(out=aggT[:, 0:P], in_=agT0[:])
                    nc.vector.tensor_copy(out=aggT[:, P : 2 * P], in_=agT1[:])

                    pm = psB.tile([P, D], f32, tag="pm", space="PSUM")
                    nc.tensor.matmul(
                        out=pm[:],
                        lhsT=aggT[:, 0:P],
                        rhs=wt_sb[:, 0:D],
                        start=True,
                        stop=False,
                    )
                    nc.tensor.matmul(
                        out=pm[:],
                        lhsT=aggT[:, P : 2 * P],
                        rhs=wt_sb[:, D : 2 * D],
                        start=False,
                        stop=False,
                    )
                    for c in range(RELCHUNKS):
                        nc.tensor.matmul(
                            out=pm[:],
                            lhsT=selr_sb[:, c * P : (c + 1) * P],
                            rhs=Rg[tg][ti // 2][
                            :,
                            ((ti % 2) * RELCHUNKS + c) * D : ((ti % 2) * RELCHUNKS + c + 1) * D,
                        ],
                            start=False,
                            stop=(c == RELCHUNKS - 1),
                        )
                    osb = small.tile([P, D], f32, tag="osb")
                    nc.scalar.activation(
                        out=osb[:], in_=pm[:], func=mybir.ActivationFunctionType.Relu
                    )
                    nc.sync.dma_start(out=out[t * P : (t + 1) * P, :], in_=osb[:])
    nc.compile()
    return nc


def _get_nc(QUOTA):
    key = ("nc", tuple(QUOTA))
    if key not in _CACHE:
        _CACHE[key] = _build(tuple(QUOTA))
    return _CACHE[key]


def _wrap16(lst, width):
    """Wrap a flat ALL-VALID index list of length width*16 into [128, width]
    int16 (16-partition wrap, replicated to all 8 gpsimd core groups).
    Negative indices + multiple in-flight gathers crash the Q7 ucode, so
    callers must pad with a valid dummy index instead."""
    n = len(lst)
    assert n == width * 16
    outw = np.asarray(lst, dtype=np.int16).reshape(width, 16).T
    return np.tile(outw, (8, 1))


def _quotas_for(neighbors):
    """Smallest 128-multiple quota per bucket covering the actual input."""
    nb = np.ascontiguousarray(neighbors, dtype=np.int64).reshape(
        N_CORES * TILES, P * S
    )
    quotas = []
    for k in range(4):
        cnts = ((nb >= WIN[k][0]) & (nb < WIN[k][1])).sum(axis=1)
        q = int(-(-max(1, cnts.max()) // P) * P)
        quotas.append(q)
    return tuple(quotas)


def _prep_inputs(neighbors, relations, features, weight, relation_weight, QUOTA):
    import ml_dtypes

    NCHUNK = [q // P for q in QUOTA]
    CHUNKS = sum(NCHUNK)

    bf16 = ml_dtypes.bfloat16
    inv_s = np.float32(1.0 / S)

    nb = np.ascontiguousarray(neighbors, dtype=np.int64).reshape(N_CORES, TILES, P, S)
    rl = np.ascontiguousarray(relations, dtype=np.int64).reshape(N_CORES, TILES, P, S)
    feat = np.ascontiguousarray(features.astype(bf16))
    rwt_s = relation_weight.T.astype(np.float32) * inv_s  # [238, 256]
    # pair table: row tri(r1,r2) = rwt_s[r1] + rwt_s[r2], r1 >= r2
    r1g, r2g = np.tril_indices(NUM_REL)
    tri = (r1g * (r1g + 1) // 2 + r2g).astype(np.int64)
    pair_tab = np.zeros((NUM_PAIR, D), dtype=np.float32)
    pair_tab[tri] = rwt_s[r1g] + rwt_s[r2g]
    rwt = np.ascontiguousarray(pair_tab.astype(bf16))
    wT = np.ascontiguousarray((weight.T.astype(np.float32) * inv_s).astype(bf16))
    iota = np.ascontiguousarray(
        np.broadcast_to(np.arange(P, dtype=np.float32), (P, P)).astype(bf16)
    )
    # pair slot j = b*8 + pairpos -> chunk c = j//128, p = j%128, b = c*16 + p//8
    selr = np.zeros((P, RELCHUNKS * P), dtype=np.float32)
    for c in range(RELCHUNKS):
        b = c * 16 + np.arange(P) // 8
        selr[np.arange(P), c * P + b] = 1.0
    selr = selr.astype(bf16)

    IDXCOLS = sum(QUOTA) // 16
    in_maps = []
    for core in range(N_CORES):
        nidx = np.zeros((P, TILES * IDXCOLS), dtype=np.int16)
        ridx = np.zeros((P, TILES * (P * S // 32)), dtype=np.int16)
        owner = np.full((P, TILES * CHUNKS), DEAD, dtype=np.float32)
        GRP = 4
        IDXG = GRP * IDXCOLS  # cols per group
        for t in range(TILES):
            tg, ti = t // GRP, t % GRP
            idxs = nb[core, t].ravel()  # j = b*16+s
            owners_flat = np.repeat(np.arange(P), S)
            goff = 0
            cbase = 0
            for k in range(4):
                m = (idxs >= WIN[k][0]) & (idxs < WIN[k][1])
                li = idxs[m] - WIN[k][0]
                lo = owners_flat[m]
                cnt = len(li)
                assert cnt <= QUOTA[k], f"bucket {k} overflow: {cnt} > {QUOTA[k]}"
                w = QUOTA[k] // 16
                lpad = np.zeros(QUOTA[k], dtype=np.int16)
                lpad[:cnt] = li
                c0 = tg * IDXG + goff * GRP + ti * w
                nidx[:, c0 : c0 + w] = _wrap16(lpad, w)
                # owner per slot: slot i -> (p=i%128, chunk=i//128)
                ow = np.full(QUOTA[k], DEAD, dtype=np.float32)
                ow[:cnt] = lo
                owner[
                    :, t * CHUNKS + cbase : t * CHUNKS + cbase + NCHUNK[k]
                ] = ow.reshape(NCHUNK[k], P).T
                goff += w
                cbase += NCHUNK[k]
            # relations: pairs (r[2i], r[2i+1]) -> triangular pair index
            rr = rl[core, t].reshape(P * S // 2, 2)
            hi = rr.max(axis=1)
            lo = rr.min(axis=1)
            plist = (hi * (hi + 1) // 2 + lo).astype(np.int16)
            rw = P * S // 32
            ridx[:, t * rw : (t + 1) * rw] = _wrap16(plist, rw)
        in_maps.append(
            {
                "feat": feat,
                "rwt": rwt,
                "wT": wT,
                "nidx": nidx,
                "ridx": ridx,
                "owner": owner,
                "iota": iota,
                "selr": selr,
            }
        )
    return in_maps


def run(in_maps, QUOTA, trace=False, tmpdir=None):
    from concourse.bass_utils import run_bass_kernel_spmd

    nc = _get_nc(QUOTA)
    res = run_bass_kernel_spmd(
        nc, in_maps, core_ids=list(range(N_CORES)), trace=trace, tmpdir=tmpdir
    )
    out = np.concatenate([res.results[i]["out"] for i in range(N_CORES)], axis=0)
    return out, res


def kernel(neighbors, relations, features, weight, relation_weight):
    QUOTA = _quotas_for(neighbors)
    in_maps = _prep_inputs(
        neighbors, relations, features, weight, relation_weight, QUOTA
    )
    out, _ = run(in_maps, QUOTA, trace=False)
    return out
